# revision 46
# baseline (speedup 1.0000x reference)
"""Multi-head attention (16 heads, d_model=2048, seq=2048, causal) on 8 trn2 cores.

Sharding: tensor-parallel over heads (2 heads/core). The QKV projection and
the attention are fused into ONE continuous PE stream: after each 512-column
s-block sbi of QKV lands, attention runs for head-0 query-block qb=sbi and
head-1 query-block qb=sbi-2 (head 1 lags two blocks). Head 0's per-head
AllToAll therefore fires right after the QKV stream (~50us earlier than the
two-phase baseline), hiding under head 1's remaining attention; head 1's
AllToAll hides under phase-3's head-0 contributions. The ACT-engine exps
overlap the next block's QKV matmuls and the PE stays at the 2.4 GHz
HAM-warm clock.

Math notes:
 - Softmax without max-subtraction: scores are O(1) in fp32.
 - K bias dropped (softmax-invariant); V bias folded into output bias.
 - Causality structural: strictly-upper 128x512 blocks skipped; diagonal
   blocks masked post-exp with a triangular bf16 mask on DVE.
 - All matmul operands bf16 (fp32 PSUM accumulation).

Schedule notes:
 - PSUM (8 banks) is time-shared via tag-shared tile pools: pool A holds
   3x[128,2,512]f32 slots (6 banks) cycling between the K/V/Q accumulators
   and score pairs (3-deep scores decouple the exp chain from the PE);
   pool B holds 2x[128,512]f32 slots (2 banks) cycling between V-transpose
   tiles, at accumulators and softmax denominators.
 - Iteration 0 memsets its accumulators and accumulates with start=False +
   skip_group_check so the first d-chunk can matmul per 256-column half —
   a 64KB first dependency under the ~26GB/s cold-DMA ramp (a per-half
   start=True would zero-wipe the sibling half: pending-zero marking is
   bank-granular). Its weight rows stream as per-dc 64KB pieces so each
   d-chunk's weights land just-in-time.
 - Inputs are pre-tiled on host so every DMA is contiguous with wide
   partition lines; each s-block's x is prefetched one iteration ahead.
 - Attention at-matmuls run two pipeline steps behind scores in one stream
   per iteration (crossing the h0/h1 mini-stream boundary), so the PE never
   drains mid-iteration.
 - Wo (8 MB) streams on the Sync queue interleaved between x blocks.
 - One warmup AllToAll at kernel start absorbs collective channel setup.
 - Phase 3: head-0 contributions j-outer (overlapping head-1's collective),
   head-1 block-major so readouts/stores stagger; one sacrificial block is
   computed entirely last.
"""
import sys

sys.path.insert(0, "/opt/trn_rl_repo")

import numpy as np
import ml_dtypes

import concourse.bass as bass
import concourse.tile as tile
from concourse import mybir, bacc
import concourse.bass_utils as bass_utils
from concourse.bass_utils import run_bass_kernel_spmd


def _install_axon_profile_hook():
    """Provide antenv.axon_hooks (missing from this image) so
    run_bass_kernel_spmd(trace=True) can capture NTFF profiles via the
    axon PJRT .so, and make artifact upload failures non-fatal."""
    import types
    import ctypes
    import contextlib

    if "antenv.axon_hooks" not in sys.modules:
        mod = types.ModuleType("antenv.axon_hooks")
        _hook_holder = {"hook": None}

        def set_axon_ntff_profile_hook(h):
            _hook_holder["hook"] = h

        def get_axon_ntff_profile_hook():
            return _hook_holder["hook"]

        mod.set_axon_ntff_profile_hook = set_axon_ntff_profile_hook
        mod.get_axon_ntff_profile_hook = get_axon_ntff_profile_hook
        sys.modules["antenv.axon_hooks"] = mod

        so_path = "/opt/axon/libaxon_pjrt.so"
        try:
            lib = ctypes.CDLL(so_path)
            lib.axon_start_nrt_profile.argtypes = [
                ctypes.POINTER(ctypes.c_int64), ctypes.c_size_t]
            lib.axon_start_nrt_profile.restype = ctypes.c_int64
            lib.axon_stop_nrt_profile.argtypes = [ctypes.c_char_p]
            lib.axon_stop_nrt_profile.restype = ctypes.c_int64

            @contextlib.contextmanager
            def _hook(output_dir, device_ids):
                import jax
                jax.devices()
                if device_ids:
                    ids = (ctypes.c_int64 * len(device_ids))(*device_ids)
                    rc = lib.axon_start_nrt_profile(ids, len(device_ids))
                else:
                    rc = lib.axon_start_nrt_profile(None, 0)
                if rc != 0:
                    raise RuntimeError(f"axon_start_nrt_profile rc={rc}")
                try:
                    yield
                finally:
                    n = lib.axon_stop_nrt_profile(str(output_dir).encode())
                    print(f"profile: {n} file(s) written to {output_dir}",
                          file=sys.stderr)

            set_axon_ntff_profile_hook(_hook)
        except OSError:
            pass

    if not getattr(bass_utils.upload_artifacts, "_safe", False):
        _orig_upload = bass_utils.upload_artifacts

        def _safe_upload(tmpdir):
            try:
                return _orig_upload(tmpdir)
            except Exception:
                return str(tmpdir)

        _safe_upload._safe = True
        bass_utils.upload_artifacts = _safe_upload


_install_axon_profile_hook()

F32 = mybir.dt.float32
BF16 = mybir.dt.bfloat16
AF = mybir.ActivationFunctionType

S = 2048          # sequence length
D = 2048          # d_model
H = 16            # heads
DH = 128          # head dim
NCORES = 8
HPC = H // NCORES  # heads per core = 2
EL = HPC * DH      # local embedding slice = 256
P = 128
QROWS = S // NCORES  # output rows per core = 256
INV_SQRT_DH = float(1.0 / np.sqrt(DH))

CORE_IDS = list(range(NCORES))

_CACHE = {}

# exported for test.py: BassKernelResults of the most recent kernel() call
LAST_RESULTS = None


def _build_module():
    nc = bacc.Bacc("TRN2", target_bir_lowering=False, debug=False,
                   num_devices=NCORES)

    # pre-tiled on host so every DMA is contiguous with wide partition lines:
    # x2[sbi, dc4, p, i*512+s] = x[sbi*512+s, (dc4*4+i)*128+p]
    x2_d = nc.dram_tensor("x2", [S // 512, D // 512, P, 2048], BF16,
                          kind="ExternalInput").ap()
    # w2[p, dc, e] = W[e_global, dc*128+p]
    wq_d = nc.dram_tensor("wq", [P, D // P, EL], BF16, kind="ExternalInput").ap()
    wk_d = nc.dram_tensor("wk", [P, D // P, EL], BF16, kind="ExternalInput").ap()
    wv_d = nc.dram_tensor("wv", [P, D // P, EL], BF16, kind="ExternalInput").ap()
    bq_d = nc.dram_tensor("bq", [P, HPC], F32, kind="ExternalInput").ap()
    # wo2[p, ec, f] = Wo[f, ec*128+p]
    wo_d = nc.dram_tensor("wo", [P, H, D], BF16, kind="ExternalInput").ap()
    bo_d = nc.dram_tensor("bo", [P, D], F32, kind="ExternalInput").ap()
    tri_d = nc.dram_tensor("tri", [P, P], BF16, kind="ExternalInput").ap()
    eye_d = nc.dram_tensor("eye", [P, P], BF16, kind="ExternalInput").ap()

    # bf16 output (upcast host-side)
    out_d = nc.dram_tensor("out", [QROWS, D], BF16, kind="ExternalOutput").ap()

    # per-head collective buffers: [q-shard (dest core), dh, q-within-shard]
    cc_in = [nc.dram_tensor(f"cc_in{h}", [NCORES, P, QROWS], BF16).ap()
             for h in range(HPC)]
    cc_out = [nc.dram_tensor(f"cc_out{h}", [NCORES, P, QROWS], BF16).ap()
              for h in range(HPC)]
    warm_in = nc.dram_tensor("warm_in", [NCORES, 16], BF16).ap()
    warm_out = nc.dram_tensor("warm_out", [NCORES, 16], BF16).ap()

    with tile.TileContext(nc, num_cores=NCORES) as tc:
        with (
            tc.tile_pool(name="const", bufs=1) as cpool,
            tc.tile_pool(name="qkv", bufs=1) as qkv_pool,
        ):
            ones_bf = cpool.tile([P, 1], BF16, name="ones_bf")
            nc.vector.memset(ones_bf[:], 1.0)
            tri_t = cpool.tile([P, P], BF16, name="tri_t")
            eye_t = cpool.tile([P, P], BF16, name="eye_t")
            bq_t = cpool.tile([P, HPC], F32, name="bq_t")

            # warm the collective channel while the fused phase runs
            nc.gpsimd.collective_compute(
                "AllToAll", mybir.AluOpType.bypass,
                replica_groups=[CORE_IDS],
                ins=[warm_in[:]], outs=[warm_out[:]])
            # warm the gpsimd broadcast ucode (first launch pays ~7us)
            wsrc = cpool.tile([1, 16], F32, name="wsrc")
            nc.vector.memset(wsrc[:], 1.0)
            wdst = cpool.tile([P, 16], F32, name="wdst")
            nc.gpsimd.partition_broadcast(wdst[:], wsrc[:])

            # per-head Q^T/K^T [dh, s] (bf16, Q pre-scaled by 1/sqrt(dh)) and
            # V [s-chunk-major, per-head, dh] resident in SBUF
            QT = [qkv_pool.tile([P, S], BF16, name=f"QT{h}") for h in range(HPC)]
            KT = [qkv_pool.tile([P, S], BF16, name=f"KT{h}") for h in range(HPC)]
            V_t = qkv_pool.tile([P, HPC, S // P, DH], BF16, name="V_t")

            # output-projection weights + bias + attn exchange landing tiles
            p3 = tc.alloc_tile_pool(name="p3", bufs=1)
            wo_t = p3.tile([P, H, D], BF16, name="wo_t")
            bo_t = p3.tile([P, D], F32, name="bo_t")
            aT = [p3.tile([P, NCORES, QROWS], BF16, name=f"aT{h}")
                  for h in range(HPC)]
            nc.scalar.dma_start(bo_t[:], bo_d[:])

            # ---------------- fused QKV + attention ----------------
            with (
                tc.tile_pool(name="w", bufs=1) as wpool,
                tc.tile_pool(name="xt", bufs=8) as xt_pool,
                tc.tile_pool(name="vsb", bufs=2) as vsb_pool,
                tc.tile_pool(name="pt", bufs=4) as pt_pool,
                tc.tile_pool(name="dacc", bufs=2) as dacc_pool,
                tc.tile_pool(name="nrm", bufs=2) as nrm,
                tc.tile_pool(name="psA", bufs=3, space="PSUM") as psA,
                tc.tile_pool(name="psB", bufs=2, space="PSUM") as psB,
            ):
                wq_t = wpool.tile([P, D // P, EL], BF16, name="wq_t")
                wk_t = wpool.tile([P, D // P, EL], BF16, name="wk_t")
                wv_t = wpool.tile([P, D // P, EL], BF16, name="wv_t")

                W_PAIRS = ((wv_t, wv_d), (wk_t, wk_d), (wq_t, wq_d))

                def load_w_chunk(c4):
                    for w_t, w_d in W_PAIRS:
                        dsl = slice(c4 * (D // P // 4),
                                    (c4 + 1) * (D // P // 4))
                        nc.scalar.dma_start(w_t[:, dsl, :], w_d[:, dsl, :])

                # 128-row head pieces of each weight first (scalar queue), so
                # the first matmul starts after ~250KB of DMA
                for w_t, w_d in W_PAIRS:
                    nc.scalar.dma_start(w_t[:, 0, :], w_d[:, 0, :])

                # wo chunks interleave between x blocks on the Sync queue
                WO_SCHED = {1: (0, 3), 2: (3, 6), 3: (6, 8)}

                def load_wo_chunk(g):
                    nc.sync.dma_start(
                        wo_t[:, 2 * g:2 * g + 2, :],
                        wo_d[:, 2 * g:2 * g + 2, :])

                # ---- attention mini-stream machinery ----
                at_tiles = {}
                dacc_tiles = {}
                pending = {}

                def emit_score(h, qb, u):
                    s_pair = psA.tile([P, 2, 512], F32, name="s_pair",
                                      tag="A")
                    pt = pt_pool.tile([P, 2, 512], BF16, name="pt")
                    offs = []
                    for j in (0, 1):
                        kc = 2 * u + j
                        off = max(0, kc * P - qb * 512)
                        offs.append(off)
                        nc.tensor.matmul(
                            s_pair[:, j, off:512],
                            KT[h][:, kc * P:(kc + 1) * P],
                            QT[h][:, qb * 512 + off:(qb + 1) * 512],
                            start=True, stop=True)
                    if 2 * u >= 4 * qb:
                        # diagonal pair: exp the union live window, then mask
                        # the 128-col partial strips (stale PSUM in the dead
                        # columns is O(1) from the QKV bank reuse, so exp
                        # never overflows)
                        off = offs[0]
                        nc.scalar.activation(
                            pt[:, :, off:512],
                            s_pair[:, :, off:512], AF.Exp)
                        for j in (0, 1):
                            off = offs[j]
                            nc.vector.tensor_mul(
                                pt[:, j, off:off + P],
                                pt[:, j, off:off + P], tri_t[:])
                    else:
                        nc.scalar.activation(pt[:, :, :],
                                             s_pair[:, :, :], AF.Exp)
                    # denominator partial sums on DVE (bf16)
                    for j in (0, 1):
                        off = offs[j]
                        if u == 0 and j == 0:
                            dacc_tiles[(h, qb)] = dacc_pool.tile(
                                [P, 512], BF16, name="dacc")
                            nc.vector.tensor_scalar_mul(
                                dacc_tiles[(h, qb)][:, :], pt[:, 0, :], 1.0)
                        else:
                            nc.vector.tensor_add(
                                dacc_tiles[(h, qb)][:, off:512],
                                dacc_tiles[(h, qb)][:, off:512],
                                pt[:, j, off:512])
                    pending[(h, qb, u)] = (pt, offs)

                def emit_at(h, qb, u):
                    pt, offs = pending.pop((h, qb, u))
                    nkc = 4 * (qb + 1)
                    if (h, qb) not in at_tiles:
                        at_tiles[(h, qb)] = psB.tile([P, 512], F32,
                                                     name="at_ps", tag="B")
                    at_ps = at_tiles[(h, qb)]
                    for j in (0, 1):
                        kc = 2 * u + j
                        off = offs[j]
                        st, sp = kc == 0, kc == nkc - 1
                        nc.tensor.matmul(
                            at_ps[:, off:512], V_t[:, h, kc, :],
                            pt[:, j, off:512], start=st, stop=sp)
                    if u == 2 * (qb + 1) - 1:
                        finish_head(h, qb)

                def finish_head(h, qb):
                    # partition-reduce the DVE denominator accumulator,
                    # normalize, and ship this (h, qb) to the cc buffer
                    den_t = psB.tile([P, 512], F32, name="den_ps", tag="B")
                    nc.tensor.matmul(den_t[0:1, :], ones_bf[:],
                                     dacc_tiles.pop((h, qb))[:],
                                     start=True, stop=True)
                    rd = nrm.tile([1, 512], F32, name="rd")
                    nc.vector.reciprocal_approx_fast(rd[:], den_t[0:1, :])
                    rb = nrm.tile([P, 512], F32, name="rb")
                    nc.gpsimd.partition_broadcast(rb[:], rd[:])
                    at_bf = nrm.tile([P, 512], BF16, name="at_bf")
                    at_ps = at_tiles.pop((h, qb))
                    # halves: the first cc store issues while the second
                    # half is still multiplying
                    for t in range(2):
                        q_sl = slice(t * QROWS, (t + 1) * QROWS)
                        nc.vector.tensor_mul(at_bf[:, q_sl], at_ps[:, q_sl],
                                             rb[:, q_sl])
                        nc.gpsimd.dma_start(
                            cc_in[h][2 * qb + t, :, :], at_bf[:, q_sl])

                def run_steps(steps):
                    # one software-pipelined stream: at-matmuls two steps
                    # behind scores so exps hide under PE work
                    for si, (h, qb, u) in enumerate(steps):
                        emit_score(h, qb, u)
                        if si >= 2:
                            emit_at(*steps[si - 2])
                    for s in steps[-2:]:
                        emit_at(*s)

                def issue_xt(sbi):
                    xts = []
                    for dc4 in range(D // P // 4):
                        xt = xt_pool.tile([P, 4, 512], BF16, name="xt")
                        xts.append(xt)
                        if sbi == 0 and dc4 == 0:
                            # fine-grained interleave of x slices and weight
                            # rows so the (ramp-speed ~26GB/s) first group
                            # never starves; 256-col halves make the very
                            # first matmul's dependency only 64KB
                            for i4 in range(4):
                                for hf in range(2):
                                    c0 = i4 * 512 + hf * 256
                                    nc.sync.dma_start(
                                        xt[:, i4, hf * 256:(hf + 1) * 256],
                                        x2_d[0, 0, :, c0:c0 + 256])
                            # per-dc 64KB pieces, dc-major: dc 1's weights
                            # land ~2.5us after dc 0's instead of behind a
                            # 576KB-per-weight blob at cold-DMA rate
                            for dcx in range(1, 4):
                                for w_t, w_d in W_PAIRS:
                                    nc.scalar.dma_start(w_t[:, dcx, :],
                                                        w_d[:, dcx, :])
                            nc.scalar.dma_start(tri_t[:], tri_d[:])
                            nc.scalar.dma_start(eye_t[:], eye_d[:])
                            nc.scalar.dma_start(bq_t[:], bq_d[:])
                        else:
                            nc.sync.dma_start(
                                xt[:].rearrange("p i s -> p (i s)"),
                                x2_d[sbi, dc4, :, :])
                        if sbi == 0 and dc4 < 3:
                            load_w_chunk(dc4 + 1)
                    return xts

                xts_cur = issue_xt(0)
                for sbi in range(S // 512):
                    s_sl = slice(sbi * 512, (sbi + 1) * 512)
                    kvv = psA.tile([P, HPC, 512], F32, name="kvv", tag="A")
                    kvk = psA.tile([P, HPC, 512], F32, name="kvk", tag="A")
                    qq = psA.tile([P, HPC, 512], F32, name="qq", tag="A")
                    q0 = qq[:, 0, :]
                    q1 = qq[:, 1, :]
                    if sbi == 0:
                        # iteration 0 accumulates onto memset PSUM with
                        # start=False throughout: the first d-chunk's
                        # matmuls run per 256-col half (64KB x pieces
                        # during the cold-DMA ramp), and a per-half
                        # start=True would zero-wipe the sibling half
                        # (pending-zero marking is bank-granular)
                        for t in (kvv, kvk, qq):
                            nc.vector.memset(t[:], 0.0)
                    # s-block 1's x is issued late (end of iteration 0): the
                    # DMA ramp window is aggregate-bandwidth-bound and the
                    # startup weight loads need it more
                    if 0 < sbi + 1 < S // 512:
                        xts_next = issue_xt(sbi + 1)
                    for dc4 in range(D // P // 4):
                        xt = xts_cur[dc4]
                        for i in range(4):
                            dc = dc4 * 4 + i
                            st = dc == 0 and sbi > 0
                            sp = dc == (D // P - 1)
                            sg = sbi == 0
                            if sbi == 0 and dc4 == 0:
                                for hf in range(2):
                                    c = slice(hf * 256, (hf + 1) * 256)
                                    xti = xt[:, i, c]
                                    for dst, w_t in ((kvv[:, 0, c], wv_t),
                                                     (kvk[:, 0, c], wk_t),
                                                     (q0[:, c], wq_t)):
                                        nc.tensor.matmul(
                                            dst, w_t[:, dc, 0:P], xti,
                                            start=False, stop=sp,
                                            skip_group_check=True)
                                    for dst, w_t in ((kvv[:, 1, c], wv_t),
                                                     (kvk[:, 1, c], wk_t),
                                                     (q1[:, c], wq_t)):
                                        nc.tensor.matmul(
                                            dst, w_t[:, dc, P:EL], xti,
                                            start=False, stop=sp,
                                            skip_group_check=True)
                                continue
                            xti = xt[:, i, :]
                            nc.tensor.matmul(kvv[:, 0, :], wv_t[:, dc, 0:P],
                                             xti, start=st, stop=sp,
                                             skip_group_check=sg)
                            nc.tensor.matmul(kvv[:, 1, :], wv_t[:, dc, P:EL],
                                             xti, start=st, stop=sp,
                                             skip_group_check=sg)
                            nc.tensor.matmul(kvk[:, 0, :], wk_t[:, dc, 0:P],
                                             xti, start=st, stop=sp,
                                             skip_group_check=sg)
                            nc.tensor.matmul(kvk[:, 1, :], wk_t[:, dc, P:EL],
                                             xti, start=st, stop=sp,
                                             skip_group_check=sg)
                            nc.tensor.matmul(q0[:], wq_t[:, dc, 0:P], xti,
                                             start=st, stop=sp,
                                             skip_group_check=sg)
                            nc.tensor.matmul(q1[:], wq_t[:, dc, P:EL], xti,
                                             start=st, stop=sp,
                                             skip_group_check=sg)
                    if sbi in WO_SCHED:
                        for g in range(*WO_SCHED[sbi]):
                            load_wo_chunk(g)
                    # drain K/V PSUM to SBUF (DVE) and Q via ACT (bias+scale)
                    vt_sb = vsb_pool.tile([P, HPC, 512], BF16, name="vt_sb")
                    nc.vector.tensor_scalar_mul(vt_sb[:], kvv[:], 1.0)
                    nc.vector.tensor_scalar_mul(KT[0][:, s_sl], kvk[:, 0, :], 1.0)
                    nc.vector.tensor_scalar_mul(KT[1][:, s_sl], kvk[:, 1, :], 1.0)
                    nc.scalar.activation(QT[0][:, s_sl], q0[:], AF.Identity,
                                         bias=bq_t[:, 0:1], scale=INV_SQRT_DH)
                    nc.scalar.activation(QT[1][:, s_sl], q1[:], AF.Identity,
                                         bias=bq_t[:, 1:2], scale=INV_SQRT_DH)

                    # V transposes for this s-block (PE, tiles share pool B's
                    # banks so pool A can hold 3 score slots)
                    for hl in range(HPC):
                        for c in range(4):
                            tr = psB.tile([P, P], BF16, name="tr", tag="B")
                            nc.tensor.transpose(
                                tr[:], vt_sb[:, hl, c * P:(c + 1) * P],
                                eye_t[:])
                            nc.vector.tensor_scalar_mul(
                                V_t[:, hl, sbi * 4 + c, :], tr[:], 1.0)

                    # attention: head 0 for qb=sbi, head 1 lags two blocks
                    steps = [(0, sbi, u) for u in range(2 * (sbi + 1))]
                    if sbi >= 2:
                        steps += [(1, sbi - 2, u) for u in range(2 * (sbi - 1))]
                    run_steps(steps)
                    if sbi == 0:
                        xts_next = issue_xt(1)
                    if sbi + 1 < S // 512:
                        xts_cur = xts_next

                # head-0 exchange fires now; head 1's last two query blocks
                # (and their exps) hide the collective latency
                nc.gpsimd.collective_compute(
                    "AllToAll", mybir.AluOpType.bypass,
                    replica_groups=[CORE_IDS],
                    ins=[cc_in[0][:]], outs=[cc_out[0][:]])

                run_steps([(1, qb, u) for qb in (2, 3)
                           for u in range(2 * (qb + 1))])

                nc.gpsimd.collective_compute(
                    "AllToAll", mybir.AluOpType.bypass,
                    replica_groups=[CORE_IDS],
                    ins=[cc_in[1][:]], outs=[cc_out[1][:]])

                # readbacks wait on the collectives; piecewise (per source
                # pair) so phase 3's first matmuls start sooner
                for h in range(HPC):
                    for jp in range(NCORES // 2):
                        nc.sync.dma_start(
                            aT[h][:, 2 * jp:2 * jp + 2, :],
                            cc_out[h][2 * jp:2 * jp + 2, :, :]
                            .rearrange("j p q -> p j q"))

            # ---------------- phase 3: output projection ----------------
            with (
                tc.tile_pool(name="osb", bufs=3) as osb,
                tc.tile_pool(name="ps_o", bufs=1, space="PSUM") as ps_o,
            ):
                # aT[h][p, j, q] = attn^T for global head (2j+h), own q slice
                SAC = (1, 3)  # sacrificial block: computed entirely last so
                              # its bank can host warm-PE dummies
                blocks = [(qc, fb) for qc in range(QROWS // P)
                          for fb in range(D // 512)]
                o_ps = {(0, 0): ps_o.tile([P, 512], F32, name="o_ps_0_0")}
                # head 0 (available first): j-outer so consecutive matmuls
                # share the moving operand; overlaps head 1's collective
                for j in range(NCORES):
                    for fb in range(D // 512):
                        for qc in range(QROWS // P):
                            if (qc, fb) == SAC:
                                continue
                            if (qc, fb) not in o_ps:
                                o_ps[(qc, fb)] = ps_o.tile(
                                    [P, 512], F32,
                                    name=f"o_ps_{qc}_{fb}")
                            nc.tensor.matmul(
                                o_ps[(qc, fb)][:],
                                aT[0][:, j, qc * P:(qc + 1) * P],
                                wo_t[:, 2 * j, fb * 512:(fb + 1) * 512],
                                start=(j == 0), stop=False)
                o_ps[SAC] = ps_o.tile([P, 512], F32, name="o_ps_sac")
                # head 1: block-major so each block's accumulation finishes
                # staggered and its readout/store overlaps the next block;
                # the sacrificial block runs fully (both heads) at the end
                for qc, fb in [b for b in blocks if b != SAC] + [SAC]:
                    if (qc, fb) == SAC:
                        for j in range(NCORES):
                            nc.tensor.matmul(
                                o_ps[SAC][:],
                                aT[0][:, j, qc * P:(qc + 1) * P],
                                wo_t[:, 2 * j, fb * 512:(fb + 1) * 512],
                                start=(j == 0), stop=False)
                    for j in range(NCORES):
                        nc.tensor.matmul(
                            o_ps[(qc, fb)][:],
                            aT[1][:, j, qc * P:(qc + 1) * P],
                            wo_t[:, 2 * j + 1, fb * 512:(fb + 1) * 512],
                            start=False, stop=(j == NCORES - 1))
                    o_sb = osb.tile([P, 512], BF16, name="o_sb")
                    nc.vector.tensor_add(o_sb[:], o_ps[(qc, fb)][:],
                                         bo_t[:, fb * 512:(fb + 1) * 512])
                    nc.sync.dma_start(
                        out_d[qc * P:(qc + 1) * P, fb * 512:(fb + 1) * 512],
                        o_sb[:])
            p3.release()

    nc.finalize()
    return nc


def kernel(x, mask, Wq, bq, Wk, bk, Wv, bv, Wo, bo):
    """Full-input MHA forward. Returns the full (2048, 2048) fp32 output.

    The mask input is assumed to be the strictly-upper-triangular causal mask
    the reference generates; causality is applied structurally on-device.
    """
    global LAST_RESULTS
    if "nc" not in _CACHE:
        _CACHE["nc"] = _build_module()
    nc = _CACHE["nc"]

    x = np.asarray(x, dtype=np.float32)
    Wq = np.asarray(Wq, dtype=np.float32)
    Wk = np.asarray(Wk, dtype=np.float32)
    Wv = np.asarray(Wv, dtype=np.float32)
    Wo = np.asarray(Wo, dtype=np.float32)
    bq = np.asarray(bq, dtype=np.float32)
    bv = np.asarray(bv, dtype=np.float32)
    bo = np.asarray(bo, dtype=np.float32)

    bf = ml_dtypes.bfloat16
    # x2[sbi, dc4, p, i*512+s] = x[sbi*512+s, (dc4*4+i)*128+p]
    x2 = np.ascontiguousarray(
        x.T.reshape(D // 512, 4, P, S // 512, 512)
        .transpose(3, 0, 2, 1, 4).reshape(S // 512, D // 512, P, 2048)
    ).astype(bf)
    # wo2[p, ec, f] = Wo[f, ec*128+p]
    wo2 = np.ascontiguousarray(
        Wo.T.reshape(H, P, D).transpose(1, 0, 2)).astype(bf)

    def wtile(W, e_sl):
        # w2[p, dc, e] = W[e_sl, :].T[dc*128+p, e]
        return np.ascontiguousarray(
            W[e_sl, :].T.reshape(D // P, P, EL).transpose(1, 0, 2)).astype(bf)

    # V bias folded into the output bias (softmax weights sum to 1);
    # K bias dropped entirely (softmax-invariant per-query shift)
    bo_full = bo + Wo @ bv
    bo_b = np.ascontiguousarray(np.broadcast_to(bo_full, (P, D)))
    tri = np.ascontiguousarray(np.triu(np.ones((P, P), np.float32))).astype(bf)
    eye = np.ascontiguousarray(np.eye(P, dtype=np.float32)).astype(bf)

    in_maps = []
    for c in range(NCORES):
        e_sl = slice(c * EL, (c + 1) * EL)
        in_maps.append({
            "x2": x2,
            "wq": wtile(Wq, e_sl),
            "wk": wtile(Wk, e_sl),
            "wv": wtile(Wv, e_sl),
            # bias layout [dh, head]; Q bias pre-scaled by 1/sqrt(dh)
            "bq": np.ascontiguousarray((bq[e_sl] * INV_SQRT_DH).reshape(HPC, P).T),
            "wo": wo2,
            "bo": bo_b,
            "tri": tri,
            "eye": eye,
        })

    res = run_bass_kernel_spmd(nc, in_maps, CORE_IDS)
    LAST_RESULTS = res
    return np.concatenate(
        [np.asarray(res.results[c]["out"]).astype(np.float32)
         for c in range(NCORES)], axis=0)


# revision 50
# speedup vs baseline: 1.0138x; 1.0138x over previous
"""Multi-head attention (16 heads, d_model=2048, seq=2048, causal) on 8 trn2 cores.

Sharding: tensor-parallel over heads (2 heads/core). The QKV projection and
the attention are fused into ONE continuous PE stream: after each 512-column
s-block sbi of QKV lands, attention runs for head-0 query-block qb=sbi and
head-1 query-block qb=sbi-2 (head 1 lags two blocks). Head 0's per-head
AllToAll therefore fires right after the QKV stream (~50us earlier than the
two-phase baseline), hiding under head 1's remaining attention; head 1's
AllToAll hides under phase-3's head-0 contributions. The ACT-engine exps
overlap the next block's QKV matmuls and the PE stays at the 2.4 GHz
HAM-warm clock.

Math notes:
 - Softmax without max-subtraction: scores are O(1) in fp32.
 - K bias dropped (softmax-invariant); V bias folded into output bias.
 - Causality structural: strictly-upper 128x512 blocks skipped; diagonal
   blocks masked post-exp with a triangular bf16 mask on DVE.
 - All matmul operands bf16 (fp32 PSUM accumulation).

Schedule notes:
 - PSUM (8 banks) is time-shared via tag-shared tile pools: pool A holds
   3x[128,2,512]f32 slots (6 banks) cycling between the K/V/Q accumulators
   and score pairs (3-deep scores decouple the exp chain from the PE);
   pool B holds 2x[128,512]f32 slots (2 banks) cycling between V-transpose
   tiles, at accumulators and softmax denominators.
 - Iteration 0 memsets its accumulators and accumulates with start=False +
   skip_group_check so the first d-chunk can matmul per 256-column half —
   a 64KB first dependency under the ~26GB/s cold-DMA ramp (a per-half
   start=True would zero-wipe the sibling half: pending-zero marking is
   bank-granular). Its weight rows stream as per-dc 64KB pieces so each
   d-chunk's weights land just-in-time.
 - Inputs are pre-tiled on host so every DMA is contiguous with wide
   partition lines; each s-block's x is prefetched one iteration ahead.
 - Attention at-matmuls run three pipeline steps behind scores (6 pt bufs)
   per iteration (crossing the h0/h1 mini-stream boundary), so the PE never
   drains mid-iteration.
 - Wo (8 MB) streams on the Sync queue interleaved between x blocks.
 - One warmup AllToAll at kernel start absorbs collective channel setup.
 - Phase 3: head-0 contributions j-outer (overlapping head-1's collective),
   head-1 block-major so readouts/stores stagger; one sacrificial block is
   computed entirely last.
"""
import sys

sys.path.insert(0, "/opt/trn_rl_repo")

import numpy as np
import ml_dtypes

import concourse.bass as bass
import concourse.tile as tile
from concourse import mybir, bacc
import concourse.bass_utils as bass_utils
from concourse.bass_utils import run_bass_kernel_spmd


def _install_axon_profile_hook():
    """Provide antenv.axon_hooks (missing from this image) so
    run_bass_kernel_spmd(trace=True) can capture NTFF profiles via the
    axon PJRT .so, and make artifact upload failures non-fatal."""
    import types
    import ctypes
    import contextlib

    if "antenv.axon_hooks" not in sys.modules:
        mod = types.ModuleType("antenv.axon_hooks")
        _hook_holder = {"hook": None}

        def set_axon_ntff_profile_hook(h):
            _hook_holder["hook"] = h

        def get_axon_ntff_profile_hook():
            return _hook_holder["hook"]

        mod.set_axon_ntff_profile_hook = set_axon_ntff_profile_hook
        mod.get_axon_ntff_profile_hook = get_axon_ntff_profile_hook
        sys.modules["antenv.axon_hooks"] = mod

        so_path = "/opt/axon/libaxon_pjrt.so"
        try:
            lib = ctypes.CDLL(so_path)
            lib.axon_start_nrt_profile.argtypes = [
                ctypes.POINTER(ctypes.c_int64), ctypes.c_size_t]
            lib.axon_start_nrt_profile.restype = ctypes.c_int64
            lib.axon_stop_nrt_profile.argtypes = [ctypes.c_char_p]
            lib.axon_stop_nrt_profile.restype = ctypes.c_int64

            @contextlib.contextmanager
            def _hook(output_dir, device_ids):
                import jax
                jax.devices()
                if device_ids:
                    ids = (ctypes.c_int64 * len(device_ids))(*device_ids)
                    rc = lib.axon_start_nrt_profile(ids, len(device_ids))
                else:
                    rc = lib.axon_start_nrt_profile(None, 0)
                if rc != 0:
                    raise RuntimeError(f"axon_start_nrt_profile rc={rc}")
                try:
                    yield
                finally:
                    n = lib.axon_stop_nrt_profile(str(output_dir).encode())
                    print(f"profile: {n} file(s) written to {output_dir}",
                          file=sys.stderr)

            set_axon_ntff_profile_hook(_hook)
        except OSError:
            pass

    if not getattr(bass_utils.upload_artifacts, "_safe", False):
        _orig_upload = bass_utils.upload_artifacts

        def _safe_upload(tmpdir):
            try:
                return _orig_upload(tmpdir)
            except Exception:
                return str(tmpdir)

        _safe_upload._safe = True
        bass_utils.upload_artifacts = _safe_upload


_install_axon_profile_hook()

F32 = mybir.dt.float32
BF16 = mybir.dt.bfloat16
AF = mybir.ActivationFunctionType

S = 2048          # sequence length
D = 2048          # d_model
H = 16            # heads
DH = 128          # head dim
NCORES = 8
HPC = H // NCORES  # heads per core = 2
EL = HPC * DH      # local embedding slice = 256
P = 128
QROWS = S // NCORES  # output rows per core = 256
INV_SQRT_DH = float(1.0 / np.sqrt(DH))

CORE_IDS = list(range(NCORES))

_CACHE = {}

# exported for test.py: BassKernelResults of the most recent kernel() call
LAST_RESULTS = None


def _build_module():
    nc = bacc.Bacc("TRN2", target_bir_lowering=False, debug=False,
                   num_devices=NCORES)

    # pre-tiled on host so every DMA is contiguous with wide partition lines:
    # x2[sbi, dc4, p, i*512+s] = x[sbi*512+s, (dc4*4+i)*128+p]
    x2_d = nc.dram_tensor("x2", [S // 512, D // 512, P, 2048], BF16,
                          kind="ExternalInput").ap()
    # w2[p, dc, e] = W[e_global, dc*128+p]
    wq_d = nc.dram_tensor("wq", [P, D // P, EL], BF16, kind="ExternalInput").ap()
    wk_d = nc.dram_tensor("wk", [P, D // P, EL], BF16, kind="ExternalInput").ap()
    wv_d = nc.dram_tensor("wv", [P, D // P, EL], BF16, kind="ExternalInput").ap()
    bq_d = nc.dram_tensor("bq", [P, HPC], F32, kind="ExternalInput").ap()
    # wo2[p, ec, f] = Wo[f, ec*128+p]
    wo_d = nc.dram_tensor("wo", [P, H, D], BF16, kind="ExternalInput").ap()
    bo_d = nc.dram_tensor("bo", [P, D], F32, kind="ExternalInput").ap()
    tri_d = nc.dram_tensor("tri", [P, P], BF16, kind="ExternalInput").ap()
    eye_d = nc.dram_tensor("eye", [P, P], BF16, kind="ExternalInput").ap()

    # bf16 output (upcast host-side)
    out_d = nc.dram_tensor("out", [QROWS, D], BF16, kind="ExternalOutput").ap()

    # per-head collective buffers: [q-shard (dest core), dh, q-within-shard]
    cc_in = [nc.dram_tensor(f"cc_in{h}", [NCORES, P, QROWS], BF16).ap()
             for h in range(HPC)]
    cc_out = [nc.dram_tensor(f"cc_out{h}", [NCORES, P, QROWS], BF16).ap()
              for h in range(HPC)]
    warm_in = nc.dram_tensor("warm_in", [NCORES, 16], BF16).ap()
    warm_out = nc.dram_tensor("warm_out", [NCORES, 16], BF16).ap()

    with tile.TileContext(nc, num_cores=NCORES) as tc:
        with (
            tc.tile_pool(name="const", bufs=1) as cpool,
            tc.tile_pool(name="qkv", bufs=1) as qkv_pool,
        ):
            ones_bf = cpool.tile([P, 1], BF16, name="ones_bf")
            nc.vector.memset(ones_bf[:], 1.0)
            tri_t = cpool.tile([P, P], BF16, name="tri_t")
            eye_t = cpool.tile([P, P], BF16, name="eye_t")
            bq_t = cpool.tile([P, HPC], F32, name="bq_t")

            # warm the collective channel while the fused phase runs
            nc.gpsimd.collective_compute(
                "AllToAll", mybir.AluOpType.bypass,
                replica_groups=[CORE_IDS],
                ins=[warm_in[:]], outs=[warm_out[:]])
            # warm the gpsimd broadcast ucode (first launch pays ~7us)
            wsrc = cpool.tile([1, 16], F32, name="wsrc")
            nc.vector.memset(wsrc[:], 1.0)
            wdst = cpool.tile([P, 16], F32, name="wdst")
            nc.gpsimd.partition_broadcast(wdst[:], wsrc[:])

            # per-head Q^T/K^T [dh, s] (bf16, Q pre-scaled by 1/sqrt(dh)) and
            # V [s-chunk-major, per-head, dh] resident in SBUF
            QT = [qkv_pool.tile([P, S], BF16, name=f"QT{h}") for h in range(HPC)]
            KT = [qkv_pool.tile([P, S], BF16, name=f"KT{h}") for h in range(HPC)]
            V_t = qkv_pool.tile([P, HPC, S // P, DH], BF16, name="V_t")

            # output-projection weights + bias + attn exchange landing tiles
            p3 = tc.alloc_tile_pool(name="p3", bufs=1)
            wo_t = p3.tile([P, H, D], BF16, name="wo_t")
            bo_t = p3.tile([P, D], F32, name="bo_t")
            aT = [p3.tile([P, NCORES, QROWS], BF16, name=f"aT{h}")
                  for h in range(HPC)]
            nc.scalar.dma_start(bo_t[:], bo_d[:])

            # ---------------- fused QKV + attention ----------------
            with (
                tc.tile_pool(name="w", bufs=1) as wpool,
                tc.tile_pool(name="xt", bufs=8) as xt_pool,
                tc.tile_pool(name="vsb", bufs=2) as vsb_pool,
                tc.tile_pool(name="pt", bufs=6) as pt_pool,
                tc.tile_pool(name="dacc", bufs=2) as dacc_pool,
                tc.tile_pool(name="nrm", bufs=2) as nrm,
                tc.tile_pool(name="psA", bufs=3, space="PSUM") as psA,
                tc.tile_pool(name="psB", bufs=2, space="PSUM") as psB,
            ):
                wq_t = wpool.tile([P, D // P, EL], BF16, name="wq_t")
                wk_t = wpool.tile([P, D // P, EL], BF16, name="wk_t")
                wv_t = wpool.tile([P, D // P, EL], BF16, name="wv_t")

                W_PAIRS = ((wv_t, wv_d), (wk_t, wk_d), (wq_t, wq_d))

                def load_w_chunk(c4):
                    for w_t, w_d in W_PAIRS:
                        dsl = slice(c4 * (D // P // 4),
                                    (c4 + 1) * (D // P // 4))
                        nc.scalar.dma_start(w_t[:, dsl, :], w_d[:, dsl, :])

                # 128-row head pieces of each weight first (scalar queue), so
                # the first matmul starts after ~250KB of DMA
                for w_t, w_d in W_PAIRS:
                    nc.scalar.dma_start(w_t[:, 0, :], w_d[:, 0, :])

                # wo chunks interleave between x blocks on the Sync queue
                WO_SCHED = {1: (0, 3), 2: (3, 6), 3: (6, 8)}

                def load_wo_chunk(g):
                    nc.sync.dma_start(
                        wo_t[:, 2 * g:2 * g + 2, :],
                        wo_d[:, 2 * g:2 * g + 2, :])

                # ---- attention mini-stream machinery ----
                at_tiles = {}
                dacc_tiles = {}
                pending = {}

                def emit_score(h, qb, u):
                    s_pair = psA.tile([P, 2, 512], F32, name="s_pair",
                                      tag="A")
                    pt = pt_pool.tile([P, 2, 512], BF16, name="pt")
                    offs = []
                    for j in (0, 1):
                        kc = 2 * u + j
                        off = max(0, kc * P - qb * 512)
                        offs.append(off)
                        nc.tensor.matmul(
                            s_pair[:, j, off:512],
                            KT[h][:, kc * P:(kc + 1) * P],
                            QT[h][:, qb * 512 + off:(qb + 1) * 512],
                            start=True, stop=True)
                    if 2 * u >= 4 * qb:
                        # diagonal pair: exp the union live window, then mask
                        # the 128-col partial strips (stale PSUM in the dead
                        # columns is O(1) from the QKV bank reuse, so exp
                        # never overflows)
                        off = offs[0]
                        nc.scalar.activation(
                            pt[:, :, off:512],
                            s_pair[:, :, off:512], AF.Exp)
                        for j in (0, 1):
                            off = offs[j]
                            nc.vector.tensor_mul(
                                pt[:, j, off:off + P],
                                pt[:, j, off:off + P], tri_t[:])
                    else:
                        nc.scalar.activation(pt[:, :, :],
                                             s_pair[:, :, :], AF.Exp)
                    # denominator partial sums on DVE (bf16)
                    for j in (0, 1):
                        off = offs[j]
                        if u == 0 and j == 0:
                            dacc_tiles[(h, qb)] = dacc_pool.tile(
                                [P, 512], BF16, name="dacc")
                            nc.vector.tensor_scalar_mul(
                                dacc_tiles[(h, qb)][:, :], pt[:, 0, :], 1.0)
                        else:
                            nc.vector.tensor_add(
                                dacc_tiles[(h, qb)][:, off:512],
                                dacc_tiles[(h, qb)][:, off:512],
                                pt[:, j, off:512])
                    pending[(h, qb, u)] = (pt, offs)

                def emit_at(h, qb, u):
                    pt, offs = pending.pop((h, qb, u))
                    nkc = 4 * (qb + 1)
                    if (h, qb) not in at_tiles:
                        at_tiles[(h, qb)] = psB.tile([P, 512], F32,
                                                     name="at_ps", tag="B")
                    at_ps = at_tiles[(h, qb)]
                    for j in (0, 1):
                        kc = 2 * u + j
                        off = offs[j]
                        st, sp = kc == 0, kc == nkc - 1
                        nc.tensor.matmul(
                            at_ps[:, off:512], V_t[:, h, kc, :],
                            pt[:, j, off:512], start=st, stop=sp)
                    if u == 2 * (qb + 1) - 1:
                        finish_head(h, qb)

                def finish_head(h, qb):
                    # partition-reduce the DVE denominator accumulator,
                    # normalize, and ship this (h, qb) to the cc buffer
                    den_t = psB.tile([P, 512], F32, name="den_ps", tag="B")
                    nc.tensor.matmul(den_t[0:1, :], ones_bf[:],
                                     dacc_tiles.pop((h, qb))[:],
                                     start=True, stop=True)
                    rd = nrm.tile([1, 512], F32, name="rd")
                    nc.vector.reciprocal_approx_fast(rd[:], den_t[0:1, :])
                    rb = nrm.tile([P, 512], F32, name="rb")
                    nc.gpsimd.partition_broadcast(rb[:], rd[:])
                    at_bf = nrm.tile([P, 512], BF16, name="at_bf")
                    at_ps = at_tiles.pop((h, qb))
                    # halves: the first cc store issues while the second
                    # half is still multiplying
                    for t in range(2):
                        q_sl = slice(t * QROWS, (t + 1) * QROWS)
                        nc.vector.tensor_mul(at_bf[:, q_sl], at_ps[:, q_sl],
                                             rb[:, q_sl])
                        nc.gpsimd.dma_start(
                            cc_in[h][2 * qb + t, :, :], at_bf[:, q_sl])

                def run_steps(steps):
                    # one software-pipelined stream: at-matmuls three steps
                    # behind scores (pool A holds 3 score slots, pt pool 6
                    # bufs) so exp latency + ACT jitter never stall the PE
                    for si, (h, qb, u) in enumerate(steps):
                        emit_score(h, qb, u)
                        if si >= 3:
                            emit_at(*steps[si - 3])
                    for s in steps[-3:]:
                        emit_at(*s)

                def issue_xt(sbi):
                    xts = []
                    for dc4 in range(D // P // 4):
                        xt = xt_pool.tile([P, 4, 512], BF16, name="xt")
                        xts.append(xt)
                        if sbi == 0 and dc4 == 0:
                            # fine-grained interleave of x slices and weight
                            # rows so the (ramp-speed ~26GB/s) first group
                            # never starves; 256-col halves make the very
                            # first matmul's dependency only 64KB
                            for i4 in range(4):
                                for hf in range(2):
                                    c0 = i4 * 512 + hf * 256
                                    nc.sync.dma_start(
                                        xt[:, i4, hf * 256:(hf + 1) * 256],
                                        x2_d[0, 0, :, c0:c0 + 256])
                            # per-dc 64KB pieces, dc-major: dc 1's weights
                            # land ~2.5us after dc 0's instead of behind a
                            # 576KB-per-weight blob at cold-DMA rate
                            for dcx in range(1, 4):
                                for w_t, w_d in W_PAIRS:
                                    nc.scalar.dma_start(w_t[:, dcx, :],
                                                        w_d[:, dcx, :])
                            nc.scalar.dma_start(tri_t[:], tri_d[:])
                            nc.scalar.dma_start(eye_t[:], eye_d[:])
                            nc.scalar.dma_start(bq_t[:], bq_d[:])
                        else:
                            nc.sync.dma_start(
                                xt[:].rearrange("p i s -> p (i s)"),
                                x2_d[sbi, dc4, :, :])
                        if sbi == 0 and dc4 < 3:
                            load_w_chunk(dc4 + 1)
                    return xts

                xts_cur = issue_xt(0)
                for sbi in range(S // 512):
                    s_sl = slice(sbi * 512, (sbi + 1) * 512)
                    kvv = psA.tile([P, HPC, 512], F32, name="kvv", tag="A")
                    kvk = psA.tile([P, HPC, 512], F32, name="kvk", tag="A")
                    qq = psA.tile([P, HPC, 512], F32, name="qq", tag="A")
                    q0 = qq[:, 0, :]
                    q1 = qq[:, 1, :]
                    if sbi == 0:
                        # iteration 0 accumulates onto memset PSUM with
                        # start=False throughout: the first d-chunk's
                        # matmuls run per 256-col half (64KB x pieces
                        # during the cold-DMA ramp), and a per-half
                        # start=True would zero-wipe the sibling half
                        # (pending-zero marking is bank-granular)
                        for t in (kvv, kvk, qq):
                            nc.vector.memset(t[:], 0.0)
                    # s-block 1's x is issued late (end of iteration 0): the
                    # DMA ramp window is aggregate-bandwidth-bound and the
                    # startup weight loads need it more
                    if 0 < sbi + 1 < S // 512:
                        xts_next = issue_xt(sbi + 1)
                    for dc4 in range(D // P // 4):
                        xt = xts_cur[dc4]
                        for i in range(4):
                            dc = dc4 * 4 + i
                            st = dc == 0 and sbi > 0
                            sp = dc == (D // P - 1)
                            sg = sbi == 0
                            if sbi == 0 and dc4 == 0:
                                for hf in range(2):
                                    c = slice(hf * 256, (hf + 1) * 256)
                                    xti = xt[:, i, c]
                                    for dst, w_t in ((kvv[:, 0, c], wv_t),
                                                     (kvk[:, 0, c], wk_t),
                                                     (q0[:, c], wq_t)):
                                        nc.tensor.matmul(
                                            dst, w_t[:, dc, 0:P], xti,
                                            start=False, stop=sp,
                                            skip_group_check=True)
                                    for dst, w_t in ((kvv[:, 1, c], wv_t),
                                                     (kvk[:, 1, c], wk_t),
                                                     (q1[:, c], wq_t)):
                                        nc.tensor.matmul(
                                            dst, w_t[:, dc, P:EL], xti,
                                            start=False, stop=sp,
                                            skip_group_check=True)
                                continue
                            xti = xt[:, i, :]
                            nc.tensor.matmul(kvv[:, 0, :], wv_t[:, dc, 0:P],
                                             xti, start=st, stop=sp,
                                             skip_group_check=sg)
                            nc.tensor.matmul(kvv[:, 1, :], wv_t[:, dc, P:EL],
                                             xti, start=st, stop=sp,
                                             skip_group_check=sg)
                            nc.tensor.matmul(kvk[:, 0, :], wk_t[:, dc, 0:P],
                                             xti, start=st, stop=sp,
                                             skip_group_check=sg)
                            nc.tensor.matmul(kvk[:, 1, :], wk_t[:, dc, P:EL],
                                             xti, start=st, stop=sp,
                                             skip_group_check=sg)
                            nc.tensor.matmul(q0[:], wq_t[:, dc, 0:P], xti,
                                             start=st, stop=sp,
                                             skip_group_check=sg)
                            nc.tensor.matmul(q1[:], wq_t[:, dc, P:EL], xti,
                                             start=st, stop=sp,
                                             skip_group_check=sg)
                    if sbi in WO_SCHED:
                        for g in range(*WO_SCHED[sbi]):
                            load_wo_chunk(g)
                    # drain K/V PSUM to SBUF (DVE) and Q via ACT (bias+scale)
                    vt_sb = vsb_pool.tile([P, HPC, 512], BF16, name="vt_sb")
                    nc.vector.tensor_scalar_mul(vt_sb[:], kvv[:], 1.0)
                    nc.vector.tensor_scalar_mul(KT[0][:, s_sl], kvk[:, 0, :], 1.0)
                    nc.vector.tensor_scalar_mul(KT[1][:, s_sl], kvk[:, 1, :], 1.0)
                    nc.scalar.activation(QT[0][:, s_sl], q0[:], AF.Identity,
                                         bias=bq_t[:, 0:1], scale=INV_SQRT_DH)
                    nc.scalar.activation(QT[1][:, s_sl], q1[:], AF.Identity,
                                         bias=bq_t[:, 1:2], scale=INV_SQRT_DH)

                    # V transposes for this s-block (PE, tiles share pool B's
                    # banks so pool A can hold 3 score slots)
                    for hl in range(HPC):
                        for c in range(4):
                            tr = psB.tile([P, P], BF16, name="tr", tag="B")
                            nc.tensor.transpose(
                                tr[:], vt_sb[:, hl, c * P:(c + 1) * P],
                                eye_t[:])
                            nc.vector.tensor_scalar_mul(
                                V_t[:, hl, sbi * 4 + c, :], tr[:], 1.0)

                    # attention: head 0 for qb=sbi, head 1 lags two blocks
                    steps = [(0, sbi, u) for u in range(2 * (sbi + 1))]
                    if sbi >= 2:
                        steps += [(1, sbi - 2, u) for u in range(2 * (sbi - 1))]
                    run_steps(steps)
                    if sbi == 0:
                        xts_next = issue_xt(1)
                    if sbi + 1 < S // 512:
                        xts_cur = xts_next

                # head-0 exchange fires now; head 1's last two query blocks
                # (and their exps) hide the collective latency
                nc.gpsimd.collective_compute(
                    "AllToAll", mybir.AluOpType.bypass,
                    replica_groups=[CORE_IDS],
                    ins=[cc_in[0][:]], outs=[cc_out[0][:]])

                run_steps([(1, qb, u) for qb in (2, 3)
                           for u in range(2 * (qb + 1))])

                nc.gpsimd.collective_compute(
                    "AllToAll", mybir.AluOpType.bypass,
                    replica_groups=[CORE_IDS],
                    ins=[cc_in[1][:]], outs=[cc_out[1][:]])

                # readbacks wait on the collectives; piecewise (per source
                # pair) so phase 3's first matmuls start sooner
                for h in range(HPC):
                    for jp in range(NCORES // 2):
                        nc.sync.dma_start(
                            aT[h][:, 2 * jp:2 * jp + 2, :],
                            cc_out[h][2 * jp:2 * jp + 2, :, :]
                            .rearrange("j p q -> p j q"))

            # ---------------- phase 3: output projection ----------------
            with (
                tc.tile_pool(name="osb", bufs=3) as osb,
                tc.tile_pool(name="ps_o", bufs=1, space="PSUM") as ps_o,
            ):
                # aT[h][p, j, q] = attn^T for global head (2j+h), own q slice
                SAC = (1, 3)  # sacrificial block: computed entirely last so
                              # its bank can host warm-PE dummies
                blocks = [(qc, fb) for qc in range(QROWS // P)
                          for fb in range(D // 512)]
                o_ps = {(0, 0): ps_o.tile([P, 512], F32, name="o_ps_0_0")}
                # head 0 (available first): j-outer so consecutive matmuls
                # share the moving operand; overlaps head 1's collective
                for j in range(NCORES):
                    for fb in range(D // 512):
                        for qc in range(QROWS // P):
                            if (qc, fb) == SAC:
                                continue
                            if (qc, fb) not in o_ps:
                                o_ps[(qc, fb)] = ps_o.tile(
                                    [P, 512], F32,
                                    name=f"o_ps_{qc}_{fb}")
                            nc.tensor.matmul(
                                o_ps[(qc, fb)][:],
                                aT[0][:, j, qc * P:(qc + 1) * P],
                                wo_t[:, 2 * j, fb * 512:(fb + 1) * 512],
                                start=(j == 0), stop=False)
                o_ps[SAC] = ps_o.tile([P, 512], F32, name="o_ps_sac")
                # head 1: block-major so each block's accumulation finishes
                # staggered and its readout/store overlaps the next block;
                # the sacrificial block runs fully (both heads) at the end
                for qc, fb in [b for b in blocks if b != SAC] + [SAC]:
                    if (qc, fb) == SAC:
                        for j in range(NCORES):
                            nc.tensor.matmul(
                                o_ps[SAC][:],
                                aT[0][:, j, qc * P:(qc + 1) * P],
                                wo_t[:, 2 * j, fb * 512:(fb + 1) * 512],
                                start=(j == 0), stop=False)
                    for j in range(NCORES):
                        nc.tensor.matmul(
                            o_ps[(qc, fb)][:],
                            aT[1][:, j, qc * P:(qc + 1) * P],
                            wo_t[:, 2 * j + 1, fb * 512:(fb + 1) * 512],
                            start=False, stop=(j == NCORES - 1))
                    o_sb = osb.tile([P, 512], BF16, name="o_sb")
                    nc.vector.tensor_add(o_sb[:], o_ps[(qc, fb)][:],
                                         bo_t[:, fb * 512:(fb + 1) * 512])
                    nc.sync.dma_start(
                        out_d[qc * P:(qc + 1) * P, fb * 512:(fb + 1) * 512],
                        o_sb[:])
            p3.release()

    nc.finalize()
    return nc


def kernel(x, mask, Wq, bq, Wk, bk, Wv, bv, Wo, bo):
    """Full-input MHA forward. Returns the full (2048, 2048) fp32 output.

    The mask input is assumed to be the strictly-upper-triangular causal mask
    the reference generates; causality is applied structurally on-device.
    """
    global LAST_RESULTS
    if "nc" not in _CACHE:
        _CACHE["nc"] = _build_module()
    nc = _CACHE["nc"]

    x = np.asarray(x, dtype=np.float32)
    Wq = np.asarray(Wq, dtype=np.float32)
    Wk = np.asarray(Wk, dtype=np.float32)
    Wv = np.asarray(Wv, dtype=np.float32)
    Wo = np.asarray(Wo, dtype=np.float32)
    bq = np.asarray(bq, dtype=np.float32)
    bv = np.asarray(bv, dtype=np.float32)
    bo = np.asarray(bo, dtype=np.float32)

    bf = ml_dtypes.bfloat16
    # x2[sbi, dc4, p, i*512+s] = x[sbi*512+s, (dc4*4+i)*128+p]
    x2 = np.ascontiguousarray(
        x.T.reshape(D // 512, 4, P, S // 512, 512)
        .transpose(3, 0, 2, 1, 4).reshape(S // 512, D // 512, P, 2048)
    ).astype(bf)
    # wo2[p, ec, f] = Wo[f, ec*128+p]
    wo2 = np.ascontiguousarray(
        Wo.T.reshape(H, P, D).transpose(1, 0, 2)).astype(bf)

    def wtile(W, e_sl):
        # w2[p, dc, e] = W[e_sl, :].T[dc*128+p, e]
        return np.ascontiguousarray(
            W[e_sl, :].T.reshape(D // P, P, EL).transpose(1, 0, 2)).astype(bf)

    # V bias folded into the output bias (softmax weights sum to 1);
    # K bias dropped entirely (softmax-invariant per-query shift)
    bo_full = bo + Wo @ bv
    bo_b = np.ascontiguousarray(np.broadcast_to(bo_full, (P, D)))
    tri = np.ascontiguousarray(np.triu(np.ones((P, P), np.float32))).astype(bf)
    eye = np.ascontiguousarray(np.eye(P, dtype=np.float32)).astype(bf)

    in_maps = []
    for c in range(NCORES):
        e_sl = slice(c * EL, (c + 1) * EL)
        in_maps.append({
            "x2": x2,
            "wq": wtile(Wq, e_sl),
            "wk": wtile(Wk, e_sl),
            "wv": wtile(Wv, e_sl),
            # bias layout [dh, head]; Q bias pre-scaled by 1/sqrt(dh)
            "bq": np.ascontiguousarray((bq[e_sl] * INV_SQRT_DH).reshape(HPC, P).T),
            "wo": wo2,
            "bo": bo_b,
            "tri": tri,
            "eye": eye,
        })

    res = run_bass_kernel_spmd(nc, in_maps, CORE_IDS)
    LAST_RESULTS = res
    return np.concatenate(
        [np.asarray(res.results[c]["out"]).astype(np.float32)
         for c in range(NCORES)], axis=0)
